# revision 2
# baseline (speedup 1.0000x reference)
"""EdgeDegreeEmbedding Trainium2 kernel (8 NeuronCores, SPMD, no collectives).

Strategy: shard by TARGET NODE (625 nodes/core). Host sorts edges by target
node and packs each node's first 16 edges into a 16-row "half"; two halves
form a 32-partition-aligned slot, 8 halves form a 128-edge MLP tile with no
padding columns. A node's message sum is computed by 7 PSUM-accumulated
matmuls (one per m-coefficient) whose stationary operand is a [32,128] slice
of the MLP output m0 and whose moving operand is a host-built block-diagonal
wigner slice [32, 98] (envelope/RESCALE pre-folded) - so the edge->node
scatter-add happens inside the PE with no data reshuffling. Nodes with more
than 16 edges spill into overflow halves that the host adds back at the end.
Each core only touches its private node range -> per-core outputs are
disjoint shards, no allreduce.

The rotation result lands transposed [channel, freq]; the host transposes
back. LayerNorm uses bn_stats + a quake-seeded Newton rsqrt (DVE+GpSimd) so
the scalar engine only ever loads the Silu table. The rotation phase of tile
t-1 is emitted during tile t's MLP (skewed pipeline) so the PE never stalls
on the m0 PSUM->SBUF cast.
"""

import numpy as np

import concourse.bass as bass
import concourse.mybir as mybir
from concourse import tile
from concourse.bass_utils import run_bass_kernel_spmd
from concourse.vector_clock import ScopedClock

# ---- problem constants (hardcoded; must match the reference) ----
SPHERE = 128
M0 = 7
LFULL = 49
CUTOFF = 12.0
RESCALE = 23.395238876342773
LN_EPS = 1e-5
N_NODES, N_EDGES, D_DIST = 5000, 50000, 512

N_CORES = 8
NODES_PER_CORE = N_NODES // N_CORES  # 625
HALF = 16                 # edges per node-half (one node's main capacity)
NPT = 8                   # halves (nodes) per tile
TILE_E = HALF * NPT       # 128 edges per tile, fully dense
H_MAIN = 632              # 625 real nodes + 7 dummies -> multiple of 8
T_MAIN = H_MAIN // NPT    # 79
WCOLS = M0 * 2 * LFULL    # 686: block-diagonal wigner section per tile row
XEC = 384                 # 768 fp8 x_edge features packed as 384 bf16 slots
XWF = XEC + WCOLS         # 1070
OUTF = NPT * LFULL        # 392
RMAGIC = 0x5F3759DF

BF16 = mybir.dt.bfloat16
F8 = mybir.dt.float8e4
F32 = mybir.dt.float32
I32 = mybir.dt.int32
NP_BF16 = mybir.dt.np(BF16)
NP_F8 = mybir.dt.np(F8)

_CACHE = {}
TRACE = False      # set True (e.g. from test.py) to profile the run
TRACE_KW = {}      # extra kwargs for run_bass_kernel_spmd when tracing
LAST = None        # BassKernelResults of the most recent run


class _ChunkedDrainTC(tile.TileContext):
    """Walrus here rejects >1 sync wait per instruction; spread every
    multi-wait instruction's extras over preceding same-engine nops, and do
    the same for the Tile exit-drain's global-clock waits."""

    def _lower_ordered_insts(self, ordered):
        for bb_name, insts in ordered.items():
            out = []
            for inst in insts:
                si = getattr(inst, "sync_info", None)
                waits = list(si.on_wait) if si is not None and si.on_wait else []
                if len(waits) > 1 and type(inst).__name__.startswith("Inst"):
                    for w in waits[:-1]:
                        out.append(mybir.InstNoOp(
                            name=self.nc.get_next_instruction_name(),
                            sync_info=mybir.SyncInfo(on_wait=[w], on_update=[]),
                            bass_nofuse=True,
                            engine=inst.engine,
                        ))
                    si.on_wait = waits[-1:]
                out.append(inst)
            ordered[bb_name] = out
        return super()._lower_ordered_insts(ordered)

    def _drain_and_barrier(self, tick_clock, wait_clock):
        nc = self.nc
        probe = nc.sync.nop()
        wait_clock.add_sem_waits(
            probe.ins, ScopedClock({None: tick_clock.global_clock})
        )
        si = probe.ins.sync_info
        waits = list(si.on_wait) if si and si.on_wait else []
        si.on_wait = waits[:1]
        for w in waits[1:]:
            n2 = nc.sync.nop()
            n2.ins.sync_info = mybir.SyncInfo(on_wait=[w], on_update=[])
        nc.sync.drain()
        nc.all_engine_barrier()
        popped = nc._tile_sem_poison_stack.pop()
        assert popped is self._sem_poison
        nc.clear_and_free_semaphores(list(self.sems.allocated().values()))
        nc.all_engine_barrier()


def _build_nc(T):
    """Build the SPMD Bass program for T tiles total (T_MAIN main tiles plus
    T-T_MAIN overflow tiles)."""
    T_OV = T - T_MAIN
    nc = bass.Bass("TRN2", target_bir_lowering=False, num_devices=N_CORES)

    xw = nc.dram_tensor("xw", [T, 128, XWF], BF16, kind="ExternalInput")
    w1 = nc.dram_tensor("w1", [128, 6 * 128], F8, kind="ExternalInput")
    w2 = nc.dram_tensor("w2", [128, 128], BF16, kind="ExternalInput")
    w3 = nc.dram_tensor("w3", [128, M0 * SPHERE], BF16, kind="ExternalInput")
    ident = nc.dram_tensor("ident", [128, 128], BF16, kind="ExternalInput")

    outr = nc.dram_tensor("outr", [T_MAIN, 128, OUTF], BF16,
                          kind="ExternalOutput")
    ovr = nc.dram_tensor("ovr", [T_OV, 128, OUTF], BF16,
                         kind="ExternalOutput")

    with _ChunkedDrainTC(nc) as tc:
        with (
            tc.tile_pool(name="const", bufs=1) as cpool,
            tc.tile_pool(name="xw", bufs=8) as xw_pool,
            tc.tile_pool(name="h", bufs=5) as h_pool,
            tc.tile_pool(name="m0", bufs=4) as m0_pool,
            tc.tile_pool(name="outt", bufs=4) as out_pool,
            tc.tile_pool(name="stat", bufs=8) as stat_pool,
            tc.tile_pool(name="ps", bufs=4, space="PSUM") as ps_pool,
            tc.tile_pool(name="psr", bufs=4, space="PSUM") as psr_pool,
        ):
            w1_sb = cpool.tile([128, 6 * 128], F8)
            nc.sync.dma_start(w1_sb[:], w1[:])
            w2_sb = cpool.tile([128, 128], BF16)
            nc.sync.dma_start(w2_sb[:], w2[:])
            w3_sb = cpool.tile([128, M0 * SPHERE], BF16)
            nc.sync.dma_start(w3_sb[:], w3[:])
            id_sb = cpool.tile([128, 128], BF16)
            nc.sync.dma_start(id_sb[:], ident[:])
            zero1 = cpool.tile([128, 1], F32)
            nc.vector.memset(zero1[:], 0.0)

            # HAM warm-up: ~5us of back-to-back matmuls raises the PE clock
            # gate from 1.2 GHz to 2.4 GHz; it then stays warm because the
            # kernel never leaves the PE idle for a full 3.4us window.
            warm_ps = ps_pool.tile([128, 128], F32, tag="ps")
            NWARM = 56
            for i in range(NWARM):
                nc.tensor.matmul(warm_ps[:], id_sb[:], id_sb[:],
                                 start=(i == 0), stop=(i == NWARM - 1))

            def layernorm_silu2(pss, h_outs):
                """h_outs[i] = silu(LN(pss[i])) for a GROUP of [128,128] f32
                psum views; one shared quake-Newton chain on [128,G]."""
                n = len(pss)
                st = stat_pool.tile([128, 6 * 4], F32, tag="bn")
                mv = stat_pool.tile([128, 2 * 4], F32, tag="mv")
                for i, ps in enumerate(pss):
                    nc.vector.bn_stats(st[:, 6 * i:6 * i + 6], ps)
                for i in range(n):
                    nc.vector.bn_aggr(mv[:, 2 * i:2 * i + 2],
                                      st[:, 6 * i:6 * i + 6])
                mvr = mv[:].rearrange("p (g v) -> p g v", v=2)
                ve = stat_pool.tile([128, 4], F32, tag="ve")
                nc.vector.tensor_scalar(ve[:, 0:n], mvr[:, 0:n, 1:2], LN_EPS,
                                        None, mybir.AluOpType.add)
                yi = stat_pool.tile([128, 4], I32, tag="yi")
                yf = yi[:].bitcast(F32)
                nc.vector.tensor_scalar(yi[:, 0:n], ve[:, 0:n].bitcast(I32),
                                        1, None,
                                        mybir.AluOpType.arith_shift_right)
                nc.vector.tensor_scalar(yi[:, 0:n], yi[:, 0:n], -1, RMAGIC,
                                        mybir.AluOpType.mult,
                                        mybir.AluOpType.add)
                t1 = stat_pool.tile([128, 4], F32, tag="t1")
                nc.gpsimd.tensor_mul(t1[:, 0:n], yf[:, 0:n], yf[:, 0:n])
                nc.gpsimd.tensor_mul(t1[:, 0:n], t1[:, 0:n], ve[:, 0:n])
                nc.vector.tensor_scalar(t1[:, 0:n], t1[:, 0:n], -0.5, 1.5,
                                        mybir.AluOpType.mult,
                                        mybir.AluOpType.add)
                nc.gpsimd.tensor_mul(yf[:, 0:n], yf[:, 0:n], t1[:, 0:n])
                nm = stat_pool.tile([128, 4], F32, tag="nm")
                nc.vector.scalar_tensor_tensor(nm[:, 0:n], mvr[:, 0:n, 0:1],
                                               -1.0, yf[:, 0:n],
                                               mybir.AluOpType.mult,
                                               mybir.AluOpType.mult)
                for i, ps in enumerate(pss):
                    nc.scalar.activation(h_outs[i][:], ps,
                                         mybir.ActivationFunctionType.Silu,
                                         bias=nm[:, i:i + 1],
                                         scale=yf[:, i:i + 1])

            def rot_phase(p):
                """Rotation + output for a previously computed tile: per
                32-aligned slot s and m, accumulate
                rotT[c, halfcols] += m0_slice.T @ w_blockdiag."""
                t, xw_t, m0_sb = p
                is_main = t < T_MAIN
                out_sb = out_pool.tile([128, OUTF], BF16)
                for s in range(4):
                    pb = 32 * s
                    rot = psr_pool.tile([128, 98], F32, tag="rot")
                    for m in range(M0):
                        nc.tensor.matmul(
                            rot[:],
                            m0_sb[pb:pb + 32, m * 128:(m + 1) * 128],
                            xw_t[pb:pb + 32,
                                 XEC + m * 98:XEC + (m + 1) * 98],
                            start=(m == 0), stop=(m == M0 - 1),
                            tile_position=(pb, 0),
                        )
                    if s % 2 == 0:
                        nc.vector.tensor_copy(
                            out_sb[:, s * 98:(s + 1) * 98], rot[:])
                    else:
                        nc.scalar.copy(
                            out_sb[:, s * 98:(s + 1) * 98], rot[:])
                nc.gpsimd.dma_start(outr[t] if is_main else ovr[t - T_MAIN],
                                    out_sb[:])

            prev = []
            GRP = 4
            assert T % GRP == 0
            for tp in range(T // GRP):
                xws, ps1s, h1s = [], [], []
                for i in range(GRP):
                    t = GRP * tp + i
                    xw_t = xw_pool.tile([128, XWF], BF16)
                    nc.sync.dma_start(xw_t[:], xw[t])
                    xws.append(xw_t)
                # L1 all tiles; xe section = 768 fp8 features bitcast
                # from 384 bf16 slots
                for i in range(GRP):
                    xe8 = xws[i][:, 0:XEC].bitcast(F8)
                    ps1 = ps_pool.tile([128, 448], F32, tag="ps")
                    for k in range(6):
                        nc.tensor.matmul(
                            ps1[:, 0:128],
                            xe8[:, k * 128:(k + 1) * 128],
                            w1_sb[:, k * 128:(k + 1) * 128],
                            start=(k == 0), stop=(k == 5),
                        )
                    ps1s.append(ps1)
                    h1 = h_pool.tile([128, 128], BF16, tag="h")
                    h1s.append(h1)
                layernorm_silu2([p[:, 0:128] for p in ps1s], h1s)

                h1ts, ps2s, h2s = [], [], []
                for i in range(GRP):
                    pst1 = ps_pool.tile([128, 128], BF16, tag="ps")
                    nc.tensor.transpose(pst1[:], h1s[i][:], id_sb[:])
                    h1t = h_pool.tile([128, 128], BF16, tag="ht")
                    nc.vector.tensor_copy(h1t[:], pst1[:])
                    h1ts.append(h1t)
                for i in range(GRP):
                    ps2 = ps_pool.tile([128, 448], F32, tag="ps")
                    nc.tensor.matmul(ps2[:, 0:128], h1ts[i][:], w2_sb[:],
                                     start=True, stop=True)
                    ps2s.append(ps2)
                    h2 = h_pool.tile([128, 128], BF16, tag="h")
                    h2s.append(h2)
                layernorm_silu2([p[:, 0:128] for p in ps2s], h2s)

                new_prev = []
                for i in range(GRP):
                    t = GRP * tp + i
                    pst2 = ps_pool.tile([128, 128], BF16, tag="ps")
                    nc.tensor.transpose(pst2[:], h2s[i][:], id_sb[:])
                    h2t = h_pool.tile([128, 128], BF16, tag="ht")
                    nc.vector.tensor_copy(h2t[:], pst2[:])

                    m0a = ps_pool.tile([128, 448], F32, tag="ps")
                    nc.tensor.matmul(m0a[:], h2t[:], w3_sb[:, 0:448],
                                     start=True, stop=True)
                    m0b = ps_pool.tile([128, 448], F32, tag="ps")
                    nc.tensor.matmul(m0b[:], h2t[:], w3_sb[:, 448:896],
                                     start=True, stop=True)
                    m0_sb = m0_pool.tile([128, M0 * SPHERE], BF16)
                    nc.scalar.activation(m0_sb[:, 0:448], m0a[:],
                                         mybir.ActivationFunctionType.Copy)
                    nc.vector.tensor_copy(m0_sb[:, 448:896], m0b[:])
                    new_prev.append((t, xws[i], m0_sb))

                # pair-skewed pipeline: rotation of the PREVIOUS pair runs
                # while this pair's MLP streams
                for p in prev:
                    rot_phase(p)
                prev = new_prev
            for p in prev:
                rot_phase(p)

    return nc


def _envelope(d):
    e = 1.0 + (-21.0) * d ** 5 + 35.0 * d ** 6 + (-15.0) * d ** 7
    return np.where(d < 1.0, e, 0.0)


def kernel(**inputs):
    x = np.asarray(inputs["x"], np.float32)
    dist_emb = np.asarray(inputs["edge_distance_embedding"], np.float32)
    src_emb = np.asarray(inputs["source_atom_embedding"], np.float32)
    tgt_emb = np.asarray(inputs["target_atom_embedding"], np.float32)
    edge_distance = np.asarray(inputs["edge_distance"], np.float64)
    edge_index = np.asarray(inputs["edge_index"]).astype(np.int64)
    wigner = np.asarray(inputs["wigner_and_M_mapping_inv"], np.float32)
    W1 = np.asarray(inputs["W1"], np.float32)
    W2 = np.asarray(inputs["W2"], np.float32)
    W3 = np.asarray(inputs["W3"], np.float32)
    # biases/gains are zeros/ones by construction; folded out of the kernel
    for nm, triv in (("b1", 0), ("bt1", 0), ("b2", 0), ("bt2", 0), ("b3", 0),
                     ("g1", 1), ("g2", 1)):
        v = np.asarray(inputs[nm])
        assert np.all(v == triv), f"{nm} not trivial; unsupported fast path"

    srcs, tgts = edge_index[0], edge_index[1]
    scale = (_envelope(edge_distance / CUTOFF) / RESCALE).astype(np.float32)

    order = np.argsort(tgts, kind="stable")
    tsorted = tgts[order]
    starts = np.searchsorted(tsorted, np.arange(N_NODES + 1))

    # ---- build halves per core (a half = <=16 edges of one node) ----
    core_halves = []
    max_ov = 0
    for c in range(N_CORES):
        halves_main = []
        halves_ov = []
        base = c * NODES_PER_CORE
        for nl in range(NODES_PER_CORE):
            eids = order[starts[base + nl]:starts[base + nl + 1]]
            halves_main.append((nl, eids[:HALF]))
            rest = eids[HALF:]
            while len(rest) > 0:
                halves_ov.append((nl, rest[:HALF]))
                rest = rest[HALF:]
        for nl in range(NODES_PER_CORE, H_MAIN):
            halves_main.append((nl, np.empty(0, np.int64)))  # dummy
        core_halves.append((halves_main, halves_ov))
        max_ov = max(max_ov, len(halves_ov))

    H_OV = max(NPT, -(-max_ov // NPT) * NPT)
    while (T_MAIN + H_OV // NPT) % 4 != 0:
        H_OV += NPT  # keep T a multiple of 4 for the batched LN loop
    H = H_MAIN + H_OV
    T = H // NPT
    E_pad = H * HALF

    if T not in _CACHE:
        _CACHE[T] = _build_nc(T)
    nc = _CACHE[T]

    # ---- shared weight tensors ----
    w1_in = np.clip(np.ascontiguousarray(
        W1.reshape(6, 128, 128).transpose(1, 0, 2).reshape(128, 6 * 128)
    ), -240.0, 240.0).astype(NP_F8)
    w2_in = W2.astype(NP_BF16)
    w3_in = W3.astype(NP_BF16)
    ident = np.eye(128, dtype=np.float32).astype(NP_BF16)

    in_maps = []
    ov_maps = []
    for c in range(N_CORES):
        halves_main, halves_ov = core_halves[c]
        halves = halves_main + halves_ov + [
            (0, np.empty(0, np.int64))
        ] * (H_OV - len(halves_ov))

        eorder = np.full(E_pad, -1, np.int64)
        for s, (_, eids) in enumerate(halves):
            eorder[s * HALF:s * HALF + len(eids)] = eids
        valid = eorder >= 0
        idx = eorder[valid]

        # xe gather -> [E_pad, 768] -> [T, 128p, 6k*128e]
        xe = np.zeros((E_pad, 768), np.float32)
        xe[valid, :D_DIST] = dist_emb[idx]
        xe[valid, D_DIST:D_DIST + 128] = src_emb[srcs[idx]]
        xe[valid, D_DIST + 128:] = tgt_emb[tgts[idx]]
        xeT = xe.reshape(T, TILE_E, 6, 128).transpose(0, 3, 2, 1)

        # block-diagonal wigner section:
        # xw[t, 32s+16h+i, 768 + m*98 + h*49 + f] = wig[e,f,m]*scale
        wrows = np.zeros((E_pad, M0, LFULL), np.float32)
        wrows[valid] = (
            wigner[idx, :, :M0] * scale[idx][:, None, None]
        ).transpose(0, 2, 1)
        wr5 = wrows.reshape(T, 4, 2, HALF, M0, LFULL)
        wsec = np.zeros((T, 4, 2, HALF, M0, 2, LFULL), np.float32)
        for h in range(2):
            wsec[:, :, h, :, :, h, :] = wr5[:, :, h]
        wsec = wsec.reshape(T, 128, WCOLS)

        xe8 = np.clip(np.ascontiguousarray(xeT.reshape(T, 128, 768)),
                      -240.0, 240.0).astype(NP_F8)
        xw_u8 = np.empty((T, 128, 2 * XWF), np.uint8)
        xw_u8[:, :, :768] = xe8.view(np.uint8)
        xw_u8[:, :, 768:] = np.ascontiguousarray(
            wsec.astype(NP_BF16)).view(np.uint8)
        xw_in = xw_u8.view(NP_BF16)

        in_maps.append({
            "xw": xw_in,
            "w1": w1_in, "w2": w2_in, "w3": w3_in, "ident": ident,
        })
        ov_maps.append([nl for nl, _ in halves_ov])

    global LAST
    res = run_bass_kernel_spmd(
        nc, in_maps, core_ids=list(range(N_CORES)), trace=TRACE, **TRACE_KW
    )
    LAST = res

    out = np.empty((N_NODES, LFULL, SPHERE), np.float32)
    for c in range(N_CORES):
        r = res.results[c]
        # [T_MAIN, 128c, 8, 49] -> [H_MAIN, 49, 128]
        o = np.asarray(r["outr"], np.float32).reshape(
            T_MAIN, 128, NPT, LFULL).transpose(0, 2, 3, 1).reshape(
            H_MAIN, LFULL, 128)
        oc = x[c * NODES_PER_CORE:(c + 1) * NODES_PER_CORE] + \
            o[:NODES_PER_CORE]
        ov = np.asarray(r["ovr"], np.float32).reshape(
            -1, 128, NPT, LFULL).transpose(0, 2, 3, 1).reshape(
            -1, LFULL, 128)
        for s, nl in enumerate(ov_maps[c]):
            oc[nl] += ov[s]
        out[c * NODES_PER_CORE:(c + 1) * NODES_PER_CORE] = oc
    return out



# revision 8
# speedup vs baseline: 1.4783x; 1.4783x over previous
"""EdgeDegreeEmbedding Trainium2 kernel (8 NeuronCores, SPMD, no collectives).

Strategy: shard by TARGET NODE (625 nodes/core). Host splits each node's
edge list into <=16-edge pieces and bin-packs pieces (<=2 per half, first-fit
decreasing) into 16-row "halves"; two halves form a 32-partition slot, 8
halves form a 128-edge MLP tile. The edge->node scatter-add happens inside
the PE: per slot s and m-coefficient, a PSUM-accumulated matmul with the fp8
MLP output m0[32s:32s+32, m] as stationary and a host-built block-diagonal
fp8 wigner section [32, 4*49] (envelope/RESCALE folded, x2^8 scaled) as
moving. Rotation matmuls are emitted m-outer/slot-inner so each LDWEIGHTS
targets a different PE row-group than the in-flight matmul and is pulled
ahead by the reorder window. Bin-packing cuts the tile count ~31% vs
one-node-per-half. Each core only touches its private node range ->
per-core outputs are disjoint shards, no allreduce.

One merged input DMA per tile and one fp8 output DMA per 4-tile group keep
the HWDGE descriptor-generation cost (~640ns per dma_start on the issuing
sequencer) off the critical path. LayerNorm uses bn_stats + a quake-seeded
Newton rsqrt (DVE+GpSimd) so the scalar engine only ever loads the Silu
table. The rotation of tile group g-1 is emitted during group g's MLP. A
warm-up burst of back-to-back matmuls raises the PE HAM clock gate at start.
"""

import numpy as np

import concourse.bass as bass
import concourse.mybir as mybir
from concourse import tile
from concourse.bass_utils import run_bass_kernel_spmd
from concourse.vector_clock import ScopedClock

# ---- problem constants (hardcoded; must match the reference) ----
SPHERE = 128
M0 = 7
LFULL = 49
CUTOFF = 12.0
RESCALE = 23.395238876342773
LN_EPS = 1e-5
N_NODES, N_EDGES, D_DIST = 5000, 50000, 512

N_CORES = 8
NODES_PER_CORE = N_NODES // N_CORES  # 625
HALF = 16                 # edge capacity of a half
NPT = 8                   # halves per tile
TILE_E = HALF * NPT       # 128 edges per tile
XEC = 384                 # 768 fp8 x_edge features packed as 384 bf16 slots
SLOTW = 4 * LFULL         # 196: output cols per 32-row slot (<=4 nodes)
WCOLS = M0 * SLOTW        # 1372 fp8 wigner cols per tile row
XWF = XEC + WCOLS // 2    # 1070 bf16 slots: 768B xe + 1372B wigner
OUTF = 4 * SLOTW          # 784 output cols per tile
WSCALE = 256.0            # wigner x2^8 on HW; host divides the output
RMAGIC = 0x5F3759DF

BF16 = mybir.dt.bfloat16
F8 = mybir.dt.float8e4
F32 = mybir.dt.float32
I32 = mybir.dt.int32
NP_BF16 = mybir.dt.np(BF16)
NP_F8 = mybir.dt.np(F8)

_CACHE = {}
TRACE = False      # set True (e.g. from test.py) to profile the run
TRACE_KW = {}      # extra kwargs for run_bass_kernel_spmd when tracing
LAST = None        # BassKernelResults of the most recent run


class _ChunkedDrainTC(tile.TileContext):
    """Walrus here rejects >1 sync wait per instruction; spread every
    multi-wait instruction's extras over preceding same-engine nops, and do
    the same for the Tile exit-drain's global-clock waits."""

    def _lower_ordered_insts(self, ordered):
        for bb_name, insts in ordered.items():
            out = []
            for inst in insts:
                si = getattr(inst, "sync_info", None)
                waits = list(si.on_wait) if si is not None and si.on_wait else []
                if len(waits) > 1 and type(inst).__name__.startswith("Inst"):
                    for w in waits[:-1]:
                        out.append(mybir.InstNoOp(
                            name=self.nc.get_next_instruction_name(),
                            sync_info=mybir.SyncInfo(on_wait=[w], on_update=[]),
                            bass_nofuse=True,
                            engine=inst.engine,
                        ))
                    si.on_wait = waits[-1:]
                out.append(inst)
            ordered[bb_name] = out
        return super()._lower_ordered_insts(ordered)

    def _drain_and_barrier(self, tick_clock, wait_clock):
        nc = self.nc
        probe = nc.sync.nop()
        wait_clock.add_sem_waits(
            probe.ins, ScopedClock({None: tick_clock.global_clock})
        )
        si = probe.ins.sync_info
        waits = list(si.on_wait) if si and si.on_wait else []
        si.on_wait = waits[:1]
        for w in waits[1:]:
            n2 = nc.sync.nop()
            n2.ins.sync_info = mybir.SyncInfo(on_wait=[w], on_update=[])
        nc.sync.drain()
        nc.all_engine_barrier()
        popped = nc._tile_sem_poison_stack.pop()
        assert popped is self._sem_poison
        nc.clear_and_free_semaphores(list(self.sems.allocated().values()))
        nc.all_engine_barrier()


def _build_nc(T):
    """Build the SPMD Bass program for T tiles."""
    nc = bass.Bass("TRN2", target_bir_lowering=False, num_devices=N_CORES)

    xw = nc.dram_tensor("xw", [T, 128, XWF], BF16, kind="ExternalInput")
    w1 = nc.dram_tensor("w1", [128, 6 * 128], F8, kind="ExternalInput")
    w2 = nc.dram_tensor("w2", [128, 128], BF16, kind="ExternalInput")
    w3 = nc.dram_tensor("w3", [128, M0 * SPHERE], BF16, kind="ExternalInput")
    ident = nc.dram_tensor("ident", [128, 128], BF16, kind="ExternalInput")

    outr = nc.dram_tensor("outr", [128, T * OUTF], F8, kind="ExternalOutput")

    GRP = 4
    assert T % GRP == 0

    with _ChunkedDrainTC(nc) as tc:
        with (
            tc.tile_pool(name="const", bufs=1) as cpool,
            tc.tile_pool(name="xw", bufs=10) as xw_pool,
            tc.tile_pool(name="h", bufs=6) as h_pool,
            tc.tile_pool(name="m0", bufs=6) as m0_pool,
            tc.tile_pool(name="gout", bufs=2) as gout_pool,
            tc.tile_pool(name="stat", bufs=8) as stat_pool,
            tc.tile_pool(name="ps", bufs=4, space="PSUM") as ps_pool,
            tc.tile_pool(name="psr", bufs=4, space="PSUM") as psr_pool,
        ):
            w1_sb = cpool.tile([128, 6 * 128], F8)
            nc.sync.dma_start(w1_sb[:], w1[:])
            w2_sb = cpool.tile([128, 128], BF16)
            nc.sync.dma_start(w2_sb[:], w2[:])
            w3_sb = cpool.tile([128, M0 * SPHERE], BF16)
            nc.sync.dma_start(w3_sb[:], w3[:])
            id_sb = cpool.tile([128, 128], BF16)
            nc.sync.dma_start(id_sb[:], ident[:])

            # HAM warm-up: ~5us of back-to-back matmuls raises the PE clock
            # gate toward 2.4 GHz before the pipelined main loop starts.
            warm_ps = ps_pool.tile([128, 448], F32, tag="ps")
            NWARM = 56
            for i in range(NWARM):
                nc.tensor.matmul(warm_ps[:, 0:128], id_sb[:], id_sb[:],
                                 start=(i == 0), stop=(i == NWARM - 1))

            def layernorm_silu2(pss, h_outs):
                """h_outs[i] = silu(LN(pss[i])) for a GROUP of [128,128] f32
                psum views; one shared quake-Newton chain on [128,G]."""
                n = len(pss)
                st = stat_pool.tile([128, 6 * 4], F32, tag="bn")
                mv = stat_pool.tile([128, 2 * 4], F32, tag="mv")
                for i, ps in enumerate(pss):
                    nc.vector.bn_stats(st[:, 6 * i:6 * i + 6], ps)
                for i in range(n):
                    nc.vector.bn_aggr(mv[:, 2 * i:2 * i + 2],
                                      st[:, 6 * i:6 * i + 6])
                mvr = mv[:].rearrange("p (g v) -> p g v", v=2)
                ve = stat_pool.tile([128, 4], F32, tag="ve")
                nc.vector.tensor_scalar(ve[:, 0:n], mvr[:, 0:n, 1:2], LN_EPS,
                                        None, mybir.AluOpType.add)
                yi = stat_pool.tile([128, 4], I32, tag="yi")
                yf = yi[:].bitcast(F32)
                nc.vector.tensor_scalar(yi[:, 0:n], ve[:, 0:n].bitcast(I32),
                                        1, None,
                                        mybir.AluOpType.arith_shift_right)
                nc.vector.tensor_scalar(yi[:, 0:n], yi[:, 0:n], -1, RMAGIC,
                                        mybir.AluOpType.mult,
                                        mybir.AluOpType.add)
                t1 = stat_pool.tile([128, 4], F32, tag="t1")
                nc.gpsimd.tensor_mul(t1[:, 0:n], yf[:, 0:n], yf[:, 0:n])
                nc.gpsimd.tensor_mul(t1[:, 0:n], t1[:, 0:n], ve[:, 0:n])
                nc.vector.tensor_scalar(t1[:, 0:n], t1[:, 0:n], -0.5, 1.5,
                                        mybir.AluOpType.mult,
                                        mybir.AluOpType.add)
                nc.gpsimd.tensor_mul(yf[:, 0:n], yf[:, 0:n], t1[:, 0:n])
                nm = stat_pool.tile([128, 4], F32, tag="nm")
                nc.vector.scalar_tensor_tensor(nm[:, 0:n], mvr[:, 0:n, 0:1],
                                               -1.0, yf[:, 0:n],
                                               mybir.AluOpType.mult,
                                               mybir.AluOpType.mult)
                for i, ps in enumerate(pss):
                    nc.scalar.activation(h_outs[i][:], ps,
                                         mybir.ActivationFunctionType.Silu,
                                         bias=nm[:, i:i + 1],
                                         scale=yf[:, i:i + 1])

            def rot_phase(p, gout):
                """Rotation for a previously computed tile into the group
                output buffer. m-outer / slot-inner emission: consecutive
                matmuls target different PE row groups, letting LDWEIGHTS
                overlap in-flight matmuls."""
                gi, xw_t, m0_sb = p
                wig8 = xw_t[:, XEC:XWF].bitcast(F8)
                rs = [psr_pool.tile([128, SLOTW], F32, tag="rot",
                                    name=f"rot{gi}_{s}")
                      for s in range(4)]
                for m in range(M0):
                    for s in range(4):
                        nc.tensor.matmul(
                            rs[s][:],
                            m0_sb[32 * s:32 * (s + 1),
                                  128 * m:128 * (m + 1)],
                            wig8[32 * s:32 * (s + 1),
                                 SLOTW * m:SLOTW * (m + 1)],
                            start=(m == 0), stop=(m == M0 - 1),
                            tile_position=(32 * s, 0),
                        )
                for s in range(4):
                    dst = gout[:, gi * OUTF + SLOTW * s:
                               gi * OUTF + SLOTW * (s + 1)]
                    if s % 2 == 0:
                        nc.vector.tensor_copy(dst, rs[s][:])
                    else:
                        nc.scalar.copy(dst, rs[s][:])

            prev = []
            prev_gout = None
            prev_tp = 0
            for tp in range(T // GRP):
                xws, ps1s, h1s = [], [], []
                for i in range(GRP):
                    t = GRP * tp + i
                    xw_t = xw_pool.tile([128, XWF], BF16)
                    nc.sync.dma_start(xw_t[:], xw[t])
                    xws.append(xw_t)
                # L1 all tiles; xe = 768 fp8 features bitcast from 384 bf16
                for i in range(GRP):
                    xe8 = xws[i][:, 0:XEC].bitcast(F8)
                    ps1 = ps_pool.tile([128, 448], F32, tag="ps")
                    for k in range(6):
                        nc.tensor.matmul(
                            ps1[:, 0:128],
                            xe8[:, k * 128:(k + 1) * 128],
                            w1_sb[:, k * 128:(k + 1) * 128],
                            start=(k == 0), stop=(k == 5),
                        )
                    ps1s.append(ps1)
                    h1 = h_pool.tile([128, 128], BF16, tag="h")
                    h1s.append(h1)
                layernorm_silu2([p[:, 0:128] for p in ps1s], h1s)

                h1ts, ps2s, h2s = [], [], []
                for i in range(GRP):
                    pst1 = ps_pool.tile([128, 128], BF16, tag="ps")
                    nc.tensor.transpose(pst1[:], h1s[i][:], id_sb[:])
                    h1t = h_pool.tile([128, 128], BF16, tag="ht")
                    nc.vector.tensor_copy(h1t[:], pst1[:])
                    h1ts.append(h1t)
                for i in range(GRP):
                    ps2 = ps_pool.tile([128, 448], F32, tag="ps")
                    nc.tensor.matmul(ps2[:, 0:128], h1ts[i][:], w2_sb[:],
                                     start=True, stop=True)
                    ps2s.append(ps2)
                    h2 = h_pool.tile([128, 128], BF16, tag="h")
                    h2s.append(h2)
                layernorm_silu2([p[:, 0:128] for p in ps2s], h2s)

                new_prev = []
                for i in range(GRP):
                    pst2 = ps_pool.tile([128, 128], BF16, tag="ps")
                    nc.tensor.transpose(pst2[:], h2s[i][:], id_sb[:])
                    h2t = h_pool.tile([128, 128], BF16, tag="ht")
                    nc.scalar.copy(h2t[:], pst2[:])

                    m0a = ps_pool.tile([128, 448], F32, tag="ps")
                    nc.tensor.matmul(m0a[:], h2t[:], w3_sb[:, 0:448],
                                     start=True, stop=True)
                    m0b = ps_pool.tile([128, 448], F32, tag="ps")
                    nc.tensor.matmul(m0b[:], h2t[:], w3_sb[:, 448:896],
                                     start=True, stop=True)
                    m0_sb = m0_pool.tile([128, M0 * SPHERE], F8)
                    nc.scalar.activation(m0_sb[:, 0:448], m0a[:],
                                         mybir.ActivationFunctionType.Copy)
                    nc.vector.tensor_copy(m0_sb[:, 448:896], m0b[:])
                    new_prev.append((i, xws[i], m0_sb))

                # group-skewed pipeline: rotation of the PREVIOUS group runs
                # while this group's MLP streams
                if prev:
                    gout = gout_pool.tile([128, GRP * OUTF], F8)
                    for p in prev:
                        rot_phase(p, gout)
                    nc.gpsimd.dma_start(
                        outr[:, prev_tp * GRP * OUTF:
                             (prev_tp + 1) * GRP * OUTF], gout[:])
                prev = new_prev
                prev_tp = tp
            gout = gout_pool.tile([128, GRP * OUTF], F8)
            for p in prev:
                rot_phase(p, gout)
            nc.gpsimd.dma_start(
                outr[:, prev_tp * GRP * OUTF:(prev_tp + 1) * GRP * OUTF],
                gout[:])

    return nc


def _envelope(d):
    e = 1.0 + (-21.0) * d ** 5 + 35.0 * d ** 6 + (-15.0) * d ** 7
    return np.where(d < 1.0, e, 0.0)


def kernel(**inputs):
    x = np.asarray(inputs["x"], np.float32)
    dist_emb = np.asarray(inputs["edge_distance_embedding"], np.float32)
    src_emb = np.asarray(inputs["source_atom_embedding"], np.float32)
    tgt_emb = np.asarray(inputs["target_atom_embedding"], np.float32)
    edge_distance = np.asarray(inputs["edge_distance"], np.float64)
    edge_index = np.asarray(inputs["edge_index"]).astype(np.int64)
    wigner = np.asarray(inputs["wigner_and_M_mapping_inv"], np.float32)
    W1 = np.asarray(inputs["W1"], np.float32)
    W2 = np.asarray(inputs["W2"], np.float32)
    W3 = np.asarray(inputs["W3"], np.float32)
    # biases/gains are zeros/ones by construction; folded out of the kernel
    for nm, triv in (("b1", 0), ("bt1", 0), ("b2", 0), ("bt2", 0), ("b3", 0),
                     ("g1", 1), ("g2", 1)):
        v = np.asarray(inputs[nm])
        assert np.all(v == triv), f"{nm} not trivial; unsupported fast path"

    srcs, tgts = edge_index[0], edge_index[1]
    scale = (_envelope(edge_distance / CUTOFF) / RESCALE).astype(np.float32)

    order = np.argsort(tgts, kind="stable")
    tsorted = tgts[order]
    starts = np.searchsorted(tsorted, np.arange(N_NODES + 1))

    # ---- per-core: split nodes into <=16-edge pieces, bin-pack into halves
    # (<=2 pieces per half, first-fit decreasing) ----
    core_bins = []
    max_halves = 0
    for c in range(N_CORES):
        base = c * NODES_PER_CORE
        pieces = []
        for nl in range(NODES_PER_CORE):
            eids = order[starts[base + nl]:starts[base + nl + 1]]
            while len(eids) > HALF:
                pieces.append((nl, eids[:HALF]))
                eids = eids[HALF:]
            if len(eids) > 0:
                pieces.append((nl, eids))
        pieces.sort(key=lambda p: -len(p[1]))
        rem, cnt, bins = [], [], []
        for p in pieces:
            n = len(p[1])
            for b in range(len(bins)):
                if rem[b] >= n and cnt[b] < 2:
                    bins[b].append(p)
                    rem[b] -= n
                    cnt[b] += 1
                    break
            else:
                bins.append([p])
                rem.append(HALF - n)
                cnt.append(1)
        core_bins.append(bins)
        max_halves = max(max_halves, len(bins))

    H = -(-max_halves // (NPT * 4)) * (NPT * 4)  # halves; T multiple of 4
    T = H // NPT
    E_pad = H * HALF

    if T not in _CACHE:
        _CACHE[T] = _build_nc(T)
    nc = _CACHE[T]

    # ---- shared weight tensors ----
    w1_in = np.clip(np.ascontiguousarray(
        W1.reshape(6, 128, 128).transpose(1, 0, 2).reshape(128, 6 * 128)
    ), -240.0, 240.0).astype(NP_F8)
    w2_in = W2.astype(NP_BF16)
    w3_in = W3.astype(NP_BF16)
    ident = np.eye(128, dtype=np.float32).astype(NP_BF16)

    in_maps = []
    piece_maps = []
    f49 = np.arange(LFULL)
    m7 = np.arange(M0)
    for c in range(N_CORES):
        bins = core_bins[c]
        # edge slot assignment + wigner column (j) per edge
        eorder = np.full(E_pad, -1, np.int64)
        slot_j = np.zeros(E_pad, np.int64)
        pieces_out = []  # (node_local, t, s, j)
        for hh, b in enumerate(bins):
            t, h_in = hh // NPT, hh % NPT
            s = h_in // 2
            off = 0
            for pi, (nl, eids) in enumerate(b):
                j = 2 * (h_in % 2) + pi
                r0 = hh * HALF + off
                eorder[r0:r0 + len(eids)] = eids
                slot_j[r0:r0 + len(eids)] = j
                off += len(eids)
                pieces_out.append((nl, t, s, j))
        valid = eorder >= 0
        idx = eorder[valid]

        # xe gather -> [E_pad, 768] -> [T, 128p, 6k*128e]
        xe = np.zeros((E_pad, 768), np.float32)
        xe[valid, :D_DIST] = dist_emb[idx]
        xe[valid, D_DIST:D_DIST + 128] = src_emb[srcs[idx]]
        xe[valid, D_DIST + 128:] = tgt_emb[tgts[idx]]
        xeT = xe.reshape(T, TILE_E, 6, 128).transpose(0, 3, 2, 1)
        xe8 = np.clip(np.ascontiguousarray(xeT.reshape(T, 128, 768)),
                      -240.0, 240.0).astype(NP_F8)

        # block-diagonal wigner: wg[row, m*196 + j*49 + f] = wig*scale*256
        wrows = (wigner[idx, :, :M0] *
                 (scale[idx] * WSCALE)[:, None, None]).transpose(0, 2, 1)
        wg7 = np.zeros((E_pad, M0, SLOTW), np.float32)
        vr = np.nonzero(valid)[0]
        jj = slot_j[vr]
        wg7[vr[:, None, None], m7[None, :, None],
            (jj[:, None, None] * LFULL + f49[None, None, :])] = wrows
        wg8 = np.clip(wg7.reshape(T, 128, WCOLS), -240.0, 240.0
                      ).astype(NP_F8)

        xw_u8 = np.empty((T, 128, 2 * XWF), np.uint8)
        xw_u8[:, :, :768] = xe8.view(np.uint8)
        xw_u8[:, :, 768:] = wg8.view(np.uint8)
        xw_in = xw_u8.view(NP_BF16)

        in_maps.append({
            "xw": xw_in,
            "w1": w1_in, "w2": w2_in, "w3": w3_in, "ident": ident,
        })
        piece_maps.append(pieces_out)

    global LAST
    res = run_bass_kernel_spmd(
        nc, in_maps, core_ids=list(range(N_CORES)), trace=TRACE, **TRACE_KW
    )
    LAST = res

    out = np.empty((N_NODES, LFULL, SPHERE), np.float32)
    inv = np.float32(1.0 / WSCALE)
    for c in range(N_CORES):
        r = res.results[c]
        # [128c, T, 4s, 4j, 49f] -> [T, 4, 4, 49, 128]
        o = (np.asarray(r["outr"]).astype(np.float32) * inv).reshape(
            128, T, 4, 4, LFULL).transpose(1, 2, 3, 4, 0)
        oc = x[c * NODES_PER_CORE:(c + 1) * NODES_PER_CORE].copy()
        for nl, t, s, j in piece_maps[c]:
            oc[nl] += o[t, s, j]
        out[c * NODES_PER_CORE:(c + 1) * NODES_PER_CORE] = oc
    return out


# revision 11
# speedup vs baseline: 1.5182x; 1.0269x over previous
"""EdgeDegreeEmbedding Trainium2 kernel (8 NeuronCores, SPMD, no collectives).

Strategy: shard by TARGET NODE (625 nodes/core). Host splits each node's
edge list into <=16-edge pieces and bin-packs pieces (<=2 per half, first-fit
decreasing) into 16-row "halves"; two halves form a 32-partition slot, 8
halves form a 128-edge MLP tile. The edge->node scatter-add happens inside
the PE: per slot s and m-coefficient, a PSUM-accumulated matmul with the fp8
MLP output m0[32s:32s+32, m] as stationary and a host-built block-diagonal
fp8 wigner section [32, N_ts] (envelope/RESCALE folded, x2^8 scaled) as
moving, where N_ts = 49 x (pieces in slot, maxed across cores) - halves are
sorted so piece counts align across cores and the matmul width is exact.
Rotation matmuls are emitted m-outer/slot-inner so each LDWEIGHTS targets a
different PE row-group than the in-flight matmul and is pulled ahead by the
reorder window; the rotation of group g-1 is emitted BETWEEN group g's MLP
phases so the PE's in-order queue always has ready work while the LayerNorm
chain (DVE/ACT) runs. Bin-packing cuts the tile count ~31% vs
one-node-per-half. Each core only touches its private node range ->
per-core outputs are disjoint shards, no allreduce.

One merged input DMA per tile and one fp8 output DMA per 4-tile group keep
the HWDGE descriptor-generation cost (~640ns per dma_start on the issuing
sequencer) off the critical path. LayerNorm uses bn_stats + a quake-seeded
Newton rsqrt (DVE+GpSimd) so the scalar engine only ever loads the Silu
table. A warm-up burst of back-to-back matmuls raises the PE HAM clock gate
at kernel start.
"""

import numpy as np

import concourse.bass as bass
import concourse.mybir as mybir
from concourse import tile
from concourse.bass_utils import run_bass_kernel_spmd
from concourse.vector_clock import ScopedClock

# ---- problem constants (hardcoded; must match the reference) ----
SPHERE = 128
M0 = 7
LFULL = 49
CUTOFF = 12.0
RESCALE = 23.395238876342773
LN_EPS = 1e-5
N_NODES, N_EDGES, D_DIST = 5000, 50000, 512

N_CORES = 8
NODES_PER_CORE = N_NODES // N_CORES  # 625
HALF = 16                 # edge capacity of a half
NPT = 8                   # halves per tile
TILE_E = HALF * NPT       # 128 edges per tile
XEC = 384                 # 768 fp8 x_edge features packed as 384 bf16 slots
MAXSLOT = 4 * LFULL       # 196: max output cols per 32-row slot (<=4 nodes)
WSCALE = 256.0            # wigner x2^8 on HW; host divides the output
RMAGIC = 0x5F3759DF
GRP = 4

BF16 = mybir.dt.bfloat16
F8 = mybir.dt.float8e4
F32 = mybir.dt.float32
I32 = mybir.dt.int32
NP_BF16 = mybir.dt.np(BF16)
NP_F8 = mybir.dt.np(F8)

_CACHE = {}
TRACE = False      # set True (e.g. from test.py) to profile the run
TRACE_KW = {}      # extra kwargs for run_bass_kernel_spmd when tracing
LAST = None        # BassKernelResults of the most recent run


class _ChunkedDrainTC(tile.TileContext):
    """Walrus here rejects >1 sync wait per instruction; spread every
    multi-wait instruction's extras over preceding same-engine nops, and do
    the same for the Tile exit-drain's global-clock waits."""

    def _lower_ordered_insts(self, ordered):
        for bb_name, insts in ordered.items():
            out = []
            for inst in insts:
                si = getattr(inst, "sync_info", None)
                waits = list(si.on_wait) if si is not None and si.on_wait else []
                if len(waits) > 1 and type(inst).__name__.startswith("Inst"):
                    for w in waits[:-1]:
                        out.append(mybir.InstNoOp(
                            name=self.nc.get_next_instruction_name(),
                            sync_info=mybir.SyncInfo(on_wait=[w], on_update=[]),
                            bass_nofuse=True,
                            engine=inst.engine,
                        ))
                    si.on_wait = waits[-1:]
                out.append(inst)
            ordered[bb_name] = out
        return super()._lower_ordered_insts(ordered)

    def _drain_and_barrier(self, tick_clock, wait_clock):
        nc = self.nc
        probe = nc.sync.nop()
        wait_clock.add_sem_waits(
            probe.ins, ScopedClock({None: tick_clock.global_clock})
        )
        si = probe.ins.sync_info
        waits = list(si.on_wait) if si and si.on_wait else []
        si.on_wait = waits[:1]
        for w in waits[1:]:
            n2 = nc.sync.nop()
            n2.ins.sync_info = mybir.SyncInfo(on_wait=[w], on_update=[])
        nc.sync.drain()
        nc.all_engine_barrier()
        popped = nc._tile_sem_poison_stack.pop()
        assert popped is self._sem_poison
        nc.clear_and_free_semaphores(list(self.sems.allocated().values()))
        nc.all_engine_barrier()


def _profile_geometry(NTS):
    """Derive per-tile layout from the slot-width profile NTS[t][s]."""
    T = len(NTS)
    nw = [max(ns) for ns in NTS]                      # wigner block width
    wcols = [M0 * w for w in nw]                      # fp8 wigner cols
    xwf = [XEC + (wc + 1) // 2 for wc in wcols]       # bf16 slots per tile
    xoff = np.concatenate([[0], np.cumsum(xwf)]).tolist()
    oc = [sum(ns) for ns in NTS]                      # out cols per tile
    ooff = np.concatenate([[0], np.cumsum(oc)]).tolist()
    return nw, wcols, xwf, xoff, oc, ooff


def _build_nc(T, NTS):
    """Build the SPMD Bass program for T tiles with slot widths NTS."""
    nc = bass.Bass("TRN2", target_bir_lowering=False, num_devices=N_CORES)
    nw, wcols, xwf, xoff, oc, ooff = _profile_geometry(NTS)

    xw = nc.dram_tensor("xw", [128, xoff[T]], BF16, kind="ExternalInput")
    w1 = nc.dram_tensor("w1", [128, 6 * 128], F8, kind="ExternalInput")
    w2 = nc.dram_tensor("w2", [128, 128], BF16, kind="ExternalInput")
    w3 = nc.dram_tensor("w3", [128, M0 * SPHERE], BF16, kind="ExternalInput")
    ident = nc.dram_tensor("ident", [128, 128], BF16, kind="ExternalInput")

    outr = nc.dram_tensor("outr", [128, ooff[T]], F8, kind="ExternalOutput")

    assert T % GRP == 0
    XWMAX = max(xwf)

    with _ChunkedDrainTC(nc) as tc:
        with (
            tc.tile_pool(name="const", bufs=1) as cpool,
            tc.tile_pool(name="xw", bufs=10) as xw_pool,
            tc.tile_pool(name="h", bufs=6) as h_pool,
            tc.tile_pool(name="m0", bufs=6) as m0_pool,
            tc.tile_pool(name="gout", bufs=2) as gout_pool,
            tc.tile_pool(name="stat", bufs=8) as stat_pool,
            tc.tile_pool(name="ps", bufs=4, space="PSUM") as ps_pool,
            tc.tile_pool(name="psr", bufs=4, space="PSUM") as psr_pool,
        ):
            w1_sb = cpool.tile([128, 6 * 128], F8)
            nc.sync.dma_start(w1_sb[:], w1[:])
            w2_sb = cpool.tile([128, 128], BF16)
            nc.sync.dma_start(w2_sb[:], w2[:])
            w3_sb = cpool.tile([128, M0 * SPHERE], BF16)
            nc.sync.dma_start(w3_sb[:], w3[:])
            id_sb = cpool.tile([128, 128], BF16)
            nc.sync.dma_start(id_sb[:], ident[:])

            # HAM warm-up: ~5us of back-to-back matmuls raises the PE clock
            # gate toward 2.4 GHz before the pipelined main loop starts.
            warm_ps = ps_pool.tile([128, 448], F32, tag="ps")
            NWARM = 56
            for i in range(NWARM):
                nc.tensor.matmul(warm_ps[:, 0:128], id_sb[:], id_sb[:],
                                 start=(i == 0), stop=(i == NWARM - 1))

            def layernorm_silu2(pss, h_outs):
                """h_outs[i] = silu(LN(pss[i])) for a GROUP of [128,128] f32
                psum views; one shared quake-Newton chain on [128,G]."""
                n = len(pss)
                st = stat_pool.tile([128, 6 * 4], F32, tag="bn")
                mv = stat_pool.tile([128, 2 * 4], F32, tag="mv")
                for i, ps in enumerate(pss):
                    nc.vector.bn_stats(st[:, 6 * i:6 * i + 6], ps)
                for i in range(n):
                    nc.vector.bn_aggr(mv[:, 2 * i:2 * i + 2],
                                      st[:, 6 * i:6 * i + 6])
                mvr = mv[:].rearrange("p (g v) -> p g v", v=2)
                ve = stat_pool.tile([128, 4], F32, tag="ve")
                nc.vector.tensor_scalar(ve[:, 0:n], mvr[:, 0:n, 1:2], LN_EPS,
                                        None, mybir.AluOpType.add)
                yi = stat_pool.tile([128, 4], I32, tag="yi")
                yf = yi[:].bitcast(F32)
                nc.vector.tensor_scalar(yi[:, 0:n], ve[:, 0:n].bitcast(I32),
                                        1, None,
                                        mybir.AluOpType.arith_shift_right)
                nc.vector.tensor_scalar(yi[:, 0:n], yi[:, 0:n], -1, RMAGIC,
                                        mybir.AluOpType.mult,
                                        mybir.AluOpType.add)
                t1 = stat_pool.tile([128, 4], F32, tag="t1")
                nc.gpsimd.tensor_mul(t1[:, 0:n], yf[:, 0:n], yf[:, 0:n])
                nc.gpsimd.tensor_mul(t1[:, 0:n], t1[:, 0:n], ve[:, 0:n])
                nc.vector.tensor_scalar(t1[:, 0:n], t1[:, 0:n], -0.5, 1.5,
                                        mybir.AluOpType.mult,
                                        mybir.AluOpType.add)
                nc.gpsimd.tensor_mul(yf[:, 0:n], yf[:, 0:n], t1[:, 0:n])
                nm = stat_pool.tile([128, 4], F32, tag="nm")
                nc.vector.scalar_tensor_tensor(nm[:, 0:n], mvr[:, 0:n, 0:1],
                                               -1.0, yf[:, 0:n],
                                               mybir.AluOpType.mult,
                                               mybir.AluOpType.mult)
                for i, ps in enumerate(pss):
                    nc.scalar.activation(h_outs[i][:], ps,
                                         mybir.ActivationFunctionType.Silu,
                                         bias=nm[:, i:i + 1],
                                         scale=yf[:, i:i + 1])

            def rot_phase(p, gout, goff):
                """Rotation for a previously computed tile into the group
                output buffer. m-outer / slot-inner emission: consecutive
                matmuls target different PE row groups, letting LDWEIGHTS
                overlap in-flight matmuls."""
                t, xw_t, m0_sb = p
                ns, w = NTS[t], nw[t]
                wig8 = xw_t[:, XEC:xwf[t]].bitcast(F8)
                rs = [psr_pool.tile([128, MAXSLOT], F32, tag="rot",
                                    name=f"rot{t % GRP}_{s}")
                      for s in range(4)]
                for m in range(M0):
                    for s in range(4):
                        if ns[s] == 0:
                            continue
                        nc.tensor.matmul(
                            rs[s][:, 0:ns[s]],
                            m0_sb[32 * s:32 * (s + 1),
                                  128 * m:128 * (m + 1)],
                            wig8[32 * s:32 * (s + 1),
                                 w * m:w * m + ns[s]],
                            start=(m == 0), stop=(m == M0 - 1),
                            tile_position=(32 * s, 0),
                        )
                off = goff
                for s in range(4):
                    if ns[s] == 0:
                        continue
                    dst = gout[:, off:off + ns[s]]
                    if s % 2 == 0:
                        nc.vector.tensor_copy(dst, rs[s][:, 0:ns[s]])
                    else:
                        nc.scalar.copy(dst, rs[s][:, 0:ns[s]])
                    off += ns[s]

            def rot_pair(prev, lo, hi, gout, gtp):
                for p in prev[lo:hi]:
                    t = p[0]
                    rot_phase(p, gout, ooff[t] - ooff[GRP * gtp])

            prev, prev_tp = [], 0
            gout = None
            for tp in range(T // GRP):
                xws, ps1s, h1s = [], [], []
                for i in range(GRP):
                    t = GRP * tp + i
                    xw_t = xw_pool.tile([128, XWMAX], BF16)
                    nc.sync.dma_start(xw_t[:, 0:xwf[t]],
                                      xw[:, xoff[t]:xoff[t + 1]])
                    xws.append(xw_t)
                # L1 all tiles; xe = 768 fp8 features bitcast from 384 bf16
                for i in range(GRP):
                    xe8 = xws[i][:, 0:XEC].bitcast(F8)
                    ps1 = ps_pool.tile([128, 448], F32, tag="ps")
                    for k in range(6):
                        nc.tensor.matmul(
                            ps1[:, 0:128],
                            xe8[:, k * 128:(k + 1) * 128],
                            w1_sb[:, k * 128:(k + 1) * 128],
                            start=(k == 0), stop=(k == 5),
                        )
                    ps1s.append(ps1)
                    h1 = h_pool.tile([128, 128], BF16, tag="h")
                    h1s.append(h1)
                if prev:
                    gout = gout_pool.tile([128, GRP * MAXSLOT * 4], F8)
                    rot_pair(prev, 0, 2, gout, prev_tp)  # fills LN1 latency
                layernorm_silu2([p[:, 0:128] for p in ps1s], h1s)

                h1ts, ps2s, h2s = [], [], []
                for i in range(GRP):
                    pst1 = ps_pool.tile([128, 128], BF16, tag="ps")
                    nc.tensor.transpose(pst1[:], h1s[i][:], id_sb[:])
                    h1t = h_pool.tile([128, 128], BF16, tag="ht")
                    nc.vector.tensor_copy(h1t[:], pst1[:])
                    h1ts.append(h1t)
                for i in range(GRP):
                    ps2 = ps_pool.tile([128, 448], F32, tag="ps")
                    nc.tensor.matmul(ps2[:, 0:128], h1ts[i][:], w2_sb[:],
                                     start=True, stop=True)
                    ps2s.append(ps2)
                    h2 = h_pool.tile([128, 128], BF16, tag="h")
                    h2s.append(h2)
                if prev:
                    rot_pair(prev, 2, 4, gout, prev_tp)  # fills LN2 latency
                    t0 = GRP * prev_tp
                    nc.gpsimd.dma_start(
                        outr[:, ooff[t0]:ooff[t0 + GRP]],
                        gout[:, 0:ooff[t0 + GRP] - ooff[t0]])
                layernorm_silu2([p[:, 0:128] for p in ps2s], h2s)

                new_prev = []
                for i in range(GRP):
                    t = GRP * tp + i
                    pst2 = ps_pool.tile([128, 128], BF16, tag="ps")
                    nc.tensor.transpose(pst2[:], h2s[i][:], id_sb[:])
                    h2t = h_pool.tile([128, 128], BF16, tag="ht")
                    nc.scalar.copy(h2t[:], pst2[:])

                    m0a = ps_pool.tile([128, 448], F32, tag="ps")
                    nc.tensor.matmul(m0a[:], h2t[:], w3_sb[:, 0:448],
                                     start=True, stop=True)
                    m0b = ps_pool.tile([128, 448], F32, tag="ps")
                    nc.tensor.matmul(m0b[:], h2t[:], w3_sb[:, 448:896],
                                     start=True, stop=True)
                    m0_sb = m0_pool.tile([128, M0 * SPHERE], F8)
                    nc.scalar.activation(m0_sb[:, 0:448], m0a[:],
                                         mybir.ActivationFunctionType.Copy)
                    nc.vector.tensor_copy(m0_sb[:, 448:896], m0b[:])
                    new_prev.append((t, xws[i], m0_sb))
                prev, prev_tp = new_prev, tp

            gout = gout_pool.tile([128, GRP * MAXSLOT * 4], F8)
            rot_pair(prev, 0, 4, gout, prev_tp)
            t0 = GRP * prev_tp
            nc.gpsimd.dma_start(outr[:, ooff[t0]:ooff[t0 + GRP]],
                                gout[:, 0:ooff[t0 + GRP] - ooff[t0]])

    return nc


def _envelope(d):
    e = 1.0 + (-21.0) * d ** 5 + 35.0 * d ** 6 + (-15.0) * d ** 7
    return np.where(d < 1.0, e, 0.0)


def kernel(**inputs):
    x = np.asarray(inputs["x"], np.float32)
    dist_emb = np.asarray(inputs["edge_distance_embedding"], np.float32)
    src_emb = np.asarray(inputs["source_atom_embedding"], np.float32)
    tgt_emb = np.asarray(inputs["target_atom_embedding"], np.float32)
    edge_distance = np.asarray(inputs["edge_distance"], np.float64)
    edge_index = np.asarray(inputs["edge_index"]).astype(np.int64)
    wigner = np.asarray(inputs["wigner_and_M_mapping_inv"], np.float32)
    W1 = np.asarray(inputs["W1"], np.float32)
    W2 = np.asarray(inputs["W2"], np.float32)
    W3 = np.asarray(inputs["W3"], np.float32)
    # biases/gains are zeros/ones by construction; folded out of the kernel
    for nm, triv in (("b1", 0), ("bt1", 0), ("b2", 0), ("bt2", 0), ("b3", 0),
                     ("g1", 1), ("g2", 1)):
        v = np.asarray(inputs[nm])
        assert np.all(v == triv), f"{nm} not trivial; unsupported fast path"

    srcs, tgts = edge_index[0], edge_index[1]
    scale = (_envelope(edge_distance / CUTOFF) / RESCALE).astype(np.float32)

    order = np.argsort(tgts, kind="stable")
    tsorted = tgts[order]
    starts = np.searchsorted(tsorted, np.arange(N_NODES + 1))

    # ---- per-core: split nodes into <=16-edge pieces, bin-pack into halves
    # (<=2 pieces per half, first-fit decreasing), sort halves so piece
    # counts align across cores ----
    core_bins = []
    max_halves = 0
    for c in range(N_CORES):
        base = c * NODES_PER_CORE
        pieces = []
        for nl in range(NODES_PER_CORE):
            eids = order[starts[base + nl]:starts[base + nl + 1]]
            while len(eids) > HALF:
                pieces.append((nl, eids[:HALF]))
                eids = eids[HALF:]
            if len(eids) > 0:
                pieces.append((nl, eids))
        pieces.sort(key=lambda p: -len(p[1]))
        rem, cnt, bins = [], [], []
        for p in pieces:
            n = len(p[1])
            for b in range(len(bins)):
                if rem[b] >= n and cnt[b] < 2:
                    bins[b].append(p)
                    rem[b] -= n
                    cnt[b] += 1
                    break
            else:
                bins.append([p])
                rem.append(HALF - n)
                cnt.append(1)
        bins.sort(key=lambda b: (-len(b), -sum(len(p[1]) for p in b)))
        core_bins.append(bins)
        max_halves = max(max_halves, len(bins))

    H = -(-max_halves // (NPT * GRP)) * (NPT * GRP)
    T = H // NPT
    E_pad = H * HALF

    # slot-width profile: pieces per slot, maxed across cores
    P = np.zeros((T, 4), np.int64)
    for c in range(N_CORES):
        pc = np.zeros((T, 4), np.int64)
        for hh, b in enumerate(core_bins[c]):
            pc[hh // NPT, (hh % NPT) // 2] += len(b)
        np.maximum(P, pc, out=P)
    NTS = tuple(tuple(int(LFULL * p) for p in row) for row in P)

    key = (T, NTS)
    if key not in _CACHE:
        _CACHE.clear()
        _CACHE[key] = _build_nc(T, NTS)
    nc = _CACHE[key]
    nw, wcols, xwf, xoff, oc, ooff = _profile_geometry(NTS)

    # ---- shared weight tensors ----
    w1_in = np.clip(np.ascontiguousarray(
        W1.reshape(6, 128, 128).transpose(1, 0, 2).reshape(128, 6 * 128)
    ), -240.0, 240.0).astype(NP_F8)
    w2_in = W2.astype(NP_BF16)
    w3_in = W3.astype(NP_BF16)
    ident = np.eye(128, dtype=np.float32).astype(NP_BF16)

    in_maps = []
    piece_maps = []
    f49 = np.arange(LFULL)
    m7 = np.arange(M0)
    for c in range(N_CORES):
        bins = core_bins[c]
        eorder = np.full(E_pad, -1, np.int64)
        slot_j = np.zeros(E_pad, np.int64)
        pieces_out = []  # (node_local, t, s, j)
        for hh, b in enumerate(bins):
            t, h_in = hh // NPT, hh % NPT
            s = h_in // 2
            # j: sequential within the slot: even-half pieces first
            jbase = len(bins[hh - 1]) if h_in % 2 == 1 else 0
            off = 0
            for pi, (nl, eids) in enumerate(b):
                j = jbase + pi
                r0 = hh * HALF + off
                eorder[r0:r0 + len(eids)] = eids
                slot_j[r0:r0 + len(eids)] = j
                off += len(eids)
                pieces_out.append((nl, t, s, j))
        valid = eorder >= 0
        idx = eorder[valid]

        # xe gather -> [E_pad, 768] -> [T, 128p, 6k*128e]
        xe = np.zeros((E_pad, 768), np.float32)
        xe[valid, :D_DIST] = dist_emb[idx]
        xe[valid, D_DIST:D_DIST + 128] = src_emb[srcs[idx]]
        xe[valid, D_DIST + 128:] = tgt_emb[tgts[idx]]
        xeT = xe.reshape(T, TILE_E, 6, 128).transpose(0, 3, 2, 1)
        xe8 = np.clip(np.ascontiguousarray(xeT.reshape(T, 128, 768)),
                      -240.0, 240.0).astype(NP_F8)

        # block-diagonal wigner, x256: per tile t cols m*nw[t] + j*49 + f
        wrows = (wigner[idx, :, :M0] *
                 (scale[idx] * WSCALE)[:, None, None]).transpose(0, 2, 1)
        wg7 = np.zeros((E_pad, M0, MAXSLOT), np.float32)
        vr = np.nonzero(valid)[0]
        jj = slot_j[vr]
        wg7[vr[:, None, None], m7[None, :, None],
            (jj[:, None, None] * LFULL + f49[None, None, :])] = wrows
        wg8 = np.clip(wg7, -240.0, 240.0).astype(NP_F8)
        wg8 = wg8.reshape(T, 128, M0, MAXSLOT)

        xw_u8 = np.zeros((128, 2 * xoff[T]), np.uint8)
        for t in range(T):
            o = 2 * xoff[t]
            xw_u8[:, o:o + 768] = xe8[t].view(np.uint8)
            wslice = wg8[t, :, :, 0:nw[t]].reshape(128, wcols[t])
            xw_u8[:, o + 768:o + 768 + wcols[t]] = wslice.view(np.uint8)
        xw_in = xw_u8.view(NP_BF16)

        in_maps.append({
            "xw": xw_in,
            "w1": w1_in, "w2": w2_in, "w3": w3_in, "ident": ident,
        })
        piece_maps.append(pieces_out)

    global LAST
    res = run_bass_kernel_spmd(
        nc, in_maps, core_ids=list(range(N_CORES)), trace=TRACE, **TRACE_KW
    )
    LAST = res

    out = np.empty((N_NODES, LFULL, SPHERE), np.float32)
    inv = np.float32(1.0 / WSCALE)
    for c in range(N_CORES):
        r = res.results[c]
        o = np.asarray(r["outr"]).astype(np.float32) * inv  # [128, TOT]
        oc_core = x[c * NODES_PER_CORE:(c + 1) * NODES_PER_CORE].copy()
        for nl, t, s, j in piece_maps[c]:
            c0 = ooff[t] + sum(NTS[t][:s]) + j * LFULL
            oc_core[nl] += o[:, c0:c0 + LFULL].T
        out[c * NODES_PER_CORE:(c + 1) * NODES_PER_CORE] = oc_core
    return out
